# revision 1
# baseline (speedup 1.0000x reference)
"""Pixelwise contrastive loss on 8 Trainium2 cores.

Phase A (per core k): indirect-gather pixel rows from a pixel-major [HW, C]
map slice (one index moves a whole 512B channel row), square+reduce on DVE,
norm via ACT sqrt (table prefetched by a dummy op), normalize muls split
ACT/DVE, emit bf16 [128, NPAD]. Host glue reassembles the global [C, 10240]
normalized matrix (the "all-gather"). Phase B (per core k): 256 pos rows x
10240 cols of cosine similarity via PE matmul in [128,2048] PSUM chunks,
one Exp per chunk on ACT; row sums via ACT accum_out for 4 chunks and DVE
tensor_reduce for 6; host computes the NLL tail in f64 from the per-row
pos/total exp sums.
"""

import sys

if "/opt/trn_rl_repo" not in sys.path:
    sys.path.insert(0, "/opt/trn_rl_repo")

import numpy as np
import ml_dtypes

from concourse import bass, mybir, bass_utils
from concourse import bacc
import concourse.tile as tile

B, C, H, W = 8, 128, 256, 256
HW = H * W
N_POS, N_NEG = 2048, 8192
NTOT = N_POS + N_NEG
NCORES = 8
BF16 = ml_dtypes.bfloat16
E1 = float(np.exp(np.float32(1.0)))

_PROG_A = {}
_PROG_B = None


def _build_phase_a(NT):
    NPAD = NT * 128
    nc = bacc.Bacc("TRN2", target_bir_lowering=False)
    mapkT = nc.dram_tensor("mapkT", [HW, C], mybir.dt.float32, kind="ExternalInput")
    tblT = nc.dram_tensor("tbl", [128, NT], mybir.dt.int32, kind="ExternalInput")
    xnT = nc.dram_tensor("xn", [128, NPAD], mybir.dt.bfloat16, kind="ExternalOutput")
    with tile.TileContext(nc) as tc:
        with tc.tile_pool(name="main", bufs=1) as pool, \
             tc.tile_pool(name="sq", bufs=2) as pool_sq:
            # sqrt-table prefetch: overlaps the ACT table load with gathers
            dmy = pool.tile([128, 1], mybir.dt.float32)
            dmy2 = pool.tile([128, 1], mybir.dt.float32)
            nc.vector.memset(dmy[:], 1.0)
            nc.scalar.sqrt(dmy2[:], dmy[:])

            tbl_s = pool.tile([128, NT], mybir.dt.int32)
            nc.sync.dma_start(out=tbl_s[:], in_=tblT[:])
            g = pool.tile([128, NPAD], mybir.dt.float32)
            n2 = pool.tile([128, NT], mybir.dt.float32)
            for t in range(NT):
                gs = g[:, t * 128:(t + 1) * 128]
                nc.gpsimd.indirect_dma_start(
                    out=gs,
                    out_offset=None,
                    in_=mapkT[:],
                    in_offset=bass.IndirectOffsetOnAxis(ap=tbl_s[:, t:t + 1], axis=0),
                )
                sq = pool_sq.tile([128, 128], mybir.dt.float32)
                nc.vector.tensor_tensor(
                    out=sq[:], in0=gs, in1=gs, op=mybir.AluOpType.mult
                )
                nc.vector.tensor_reduce(
                    out=n2[:, t:t + 1], in_=sq[:],
                    axis=mybir.AxisListType.X, op=mybir.AluOpType.add,
                )
            nrm = pool.tile([128, NT], mybir.dt.float32)
            nc.scalar.sqrt(nrm[:], n2[:])
            r1 = pool.tile([128, NT], mybir.dt.float32)
            nc.vector.reciprocal(out=r1[:], in_=nrm[:])
            # x / max(norm, 1e-6) == x * min(1/norm, 1e6)
            r3 = pool.tile([128, NT], mybir.dt.float32)
            nc.vector.tensor_scalar_min(out=r3[:], in0=r1[:], scalar1=1.0e6)
            xn = pool.tile([128, NPAD], mybir.dt.bfloat16)
            CHD = 4  # tiles per output DMA chunk
            for t in range(NT):
                xs = xn[:, t * 128:(t + 1) * 128]
                gs = g[:, t * 128:(t + 1) * 128]
                if t % 2 == 0:
                    nc.vector.tensor_scalar_mul(out=xs, in0=gs, scalar1=r3[:, t:t + 1])
                else:
                    nc.scalar.activation(
                        out=xs, in_=gs,
                        func=mybir.ActivationFunctionType.Copy,
                        scale=r3[:, t:t + 1],
                    )
                if t % CHD == CHD - 1 or t == NT - 1:
                    lo = (t // CHD) * CHD * 128
                    hi = (t + 1) * 128
                    nc.sync.dma_start(out=xnT[:, lo:hi], in_=xn[:, lo:hi])
    nc.finalize()
    return nc


def _build_phase_b():
    NCH = 5  # column chunks of 2048; chunk 0 is exactly the pos columns
    DVE_B = (1, 2, 3)  # chunks whose row sums go to DVE instead of ACT accum
    nc = bacc.Bacc("TRN2", target_bir_lowering=False)
    posT = nc.dram_tensor("posT", [128, 256], mybir.dt.bfloat16, kind="ExternalInput")
    colsT = nc.dram_tensor("cols", [128, NTOT], mybir.dt.bfloat16, kind="ExternalInput")
    sumsT = nc.dram_tensor("sums", [128, 2 * NCH], mybir.dt.float32, kind="ExternalOutput")
    with tile.TileContext(nc) as tc:
        with tc.tile_pool(name="main", bufs=1) as pool, \
             tc.tile_pool(name="ps", bufs=2, space="PSUM") as pool_ps, \
             tc.tile_pool(name="es", bufs=2) as pool_es:
            # exp-table prefetch: overlaps the ACT table load with input DMAs
            dmy = pool.tile([128, 1], mybir.dt.float32)
            dmy2 = pool.tile([128, 1], mybir.dt.float32)
            nc.vector.memset(dmy[:], 0.0)
            nc.scalar.activation(
                out=dmy2[:], in_=dmy[:], func=mybir.ActivationFunctionType.Exp
            )

            posT_s = pool.tile([128, 256], mybir.dt.bfloat16)
            nc.sync.dma_start(out=posT_s[:], in_=posT[:])
            cols_s = pool.tile([128, NTOT], mybir.dt.bfloat16)
            for b in range(NCH):
                sl = slice(b * 2048, (b + 1) * 2048)
                nc.sync.dma_start(out=cols_s[:, sl], in_=colsT[:, sl])
            sums_s = pool.tile([128, 2 * NCH], mybir.dt.float32)
            for b in range(NCH):
                for gg in range(2):
                    ps = pool_ps.tile([128, 2048], mybir.dt.float32)
                    for q in range(4):
                        nc.tensor.matmul(
                            out=ps[:, q * 512:(q + 1) * 512],
                            lhsT=posT_s[:, gg * 128:(gg + 1) * 128],
                            rhs=cols_s[:, b * 2048 + q * 512:b * 2048 + (q + 1) * 512],
                            start=True,
                            stop=True,
                        )
                    es = pool_es.tile([128, 2048], mybir.dt.float32)
                    col = sums_s[:, gg * NCH + b:gg * NCH + b + 1]
                    if b in DVE_B:
                        nc.scalar.activation(
                            out=es[:], in_=ps[:],
                            func=mybir.ActivationFunctionType.Exp,
                        )
                        nc.vector.tensor_reduce(
                            out=col, in_=es[:],
                            axis=mybir.AxisListType.X, op=mybir.AluOpType.add,
                        )
                    else:
                        nc.scalar.activation(
                            out=es[:], in_=ps[:],
                            func=mybir.ActivationFunctionType.Exp,
                            accum_out=col,
                        )
            nc.sync.dma_start(out=sumsT[:], in_=sums_s[:])
    nc.finalize()
    return nc


def _get_out(core_results, key):
    if key in core_results:
        return np.asarray(core_results[key])
    return np.asarray(next(iter(core_results.values())))


def _run_all(inputs, trace=False):
    global _PROG_B
    psm = np.asarray(inputs["predict_seg_map"], dtype=np.float32)
    pb = np.asarray(inputs["pos_b"]).astype(np.int64)
    ph = np.asarray(inputs["pos_h"]).astype(np.int64)
    pw = np.asarray(inputs["pos_w"]).astype(np.int64)
    nb = np.asarray(inputs["neg_b"]).astype(np.int64)
    nh = np.asarray(inputs["neg_h"]).astype(np.int64)
    nw = np.asarray(inputs["neg_w"]).astype(np.int64)

    allb = np.concatenate([pb, nb])
    allpix = np.concatenate([ph * W + pw, nh * W + nw])
    gids = np.arange(NTOT, dtype=np.int64)

    ids_per, pix_per = [], []
    for k in range(NCORES):
        m = allb == k
        idk, pxk = gids[m], allpix[m]
        o = np.argsort(pxk, kind="stable")
        ids_per.append(idk[o])
        pix_per.append(pxk[o])
    nmax = max(len(x) for x in ids_per)
    NT = (nmax + 127) // 128
    NPAD = NT * 128

    psmT = np.ascontiguousarray(psm.reshape(B, C, HW).transpose(0, 2, 1))
    tbls = []
    for k in range(NCORES):
        e = np.zeros(NPAD, np.int64)
        e[:len(pix_per[k])] = pix_per[k]
        tbls.append(np.ascontiguousarray(e.reshape(NT, 128).T.astype(np.int32)))

    if NT not in _PROG_A:
        _PROG_A[NT] = _build_phase_a(NT)
    nc_a = _PROG_A[NT]
    in_maps_a = [{"mapkT": psmT[k], "tbl": tbls[k]} for k in range(NCORES)]
    ra = bass_utils.run_bass_kernel_spmd(
        nc_a, in_maps_a, list(range(NCORES)), trace=trace
    )

    allN_T = np.zeros((NTOT, C), dtype=BF16)
    for k in range(NCORES):
        xnk = _get_out(ra.results[k], "xn")  # [128, NPAD]
        nk = len(ids_per[k])
        v = xnk.reshape(128, NT, 128).transpose(1, 0, 2).reshape(NPAD, 128)[:nk]
        allN_T[ids_per[k]] = v
    cols = np.ascontiguousarray(allN_T.T)  # [C, NTOT]

    if _PROG_B is None:
        _PROG_B = _build_phase_b()
    in_maps_b = [
        {
            "posT": np.ascontiguousarray(cols[:, k * 256:(k + 1) * 256]),
            "cols": cols,
        }
        for k in range(NCORES)
    ]
    rb = bass_utils.run_bass_kernel_spmd(
        _PROG_B, in_maps_b, list(range(NCORES)), trace=trace
    )

    tot = 0.0
    for k in range(NCORES):
        sums = _get_out(rb.results[k], "sums").astype(np.float64)  # [128, 10]
        for gg in range(2):
            possum = sums[:, gg * 5]
            total = sums[:, gg * 5:(gg + 1) * 5].sum(axis=1)
            tot += float(np.log((possum - E1) / (total - E1)).sum())
    nll = -tot / N_POS

    ns = None
    if trace:
        ns = (ra.exec_time_ns or 0) + (rb.exec_time_ns or 0)
    return np.float32(nll), ns


def kernel(predict_seg_map, pos_b, pos_h, pos_w, neg_b, neg_h, neg_w):
    out, _ = _run_all(
        {
            "predict_seg_map": predict_seg_map,
            "pos_b": pos_b, "pos_h": pos_h, "pos_w": pos_w,
            "neg_b": neg_b, "neg_h": neg_h, "neg_w": neg_w,
        },
        trace=False,
    )
    return np.asarray(out, dtype=np.float32)

